# revision 9
# baseline (speedup 1.0000x reference)
"""Batched LoRA embedding lookup on 8 trn2 NeuronCores.

out[m, b, t, :] = weight[x[b,t], :] + lora_B[m] @ lora_A[m, :, x[b,t]]

Sharding: the 8192 tokens (B*T) are split 1024 per core; every core computes
all M=4 ensemble members for its tokens (reads each gathered base row once,
writes 4 output rows). The frozen base table + lora tensors are replicated.

Per-core kernel (Tile framework):
  - gather 128 base rows/tile via indirect DMA  -> base_t [128, 2048]
  - gather 128 rows of host-pretransposed A_cat [V, M*R] -> a_t [128, 64]
  - PE-transpose per member -> after_A [16, 128]; matmul vs B^T [16, 2048]
  - DVE add base + delta, DMA out 1 MB per (tile, member)
"""

import numpy as np
from contextlib import ExitStack

import concourse.bass as bass
import concourse.tile as tile
from concourse import bacc, mybir
from concourse.bass_utils import run_bass_kernel_spmd
from concourse.masks import make_identity

M, R, V, D = 4, 16, 32000, 2048
B, T = 4, 2048
NCORES = 8
P = 128
NTOK = (B * T) // NCORES          # tokens per core
NTILE = NTOK // P                 # token tiles per core
DCHUNK = 512                      # matmul N per PSUM bank (fp32)

_CACHE: dict = {}


def build_bass(ntok: int = NTOK, use_f32r: bool = False):
    """Build the SPMD Bass program (identical on all cores)."""
    mm_dtype = mybir.dt.float32
    ntile = ntok // P
    nc = bacc.Bacc()
    w = nc.declare_dram_parameter("w", [V, D], mybir.dt.float32, isOutput=False)
    acat = nc.declare_dram_parameter("acat", [V, M * R], mybir.dt.float32, isOutput=False)
    bt = nc.declare_dram_parameter("bt", [M, R, D], mybir.dt.float32, isOutput=False)
    idx = nc.declare_dram_parameter("idx", [P, ntile], mybir.dt.int32, isOutput=False)
    out = nc.declare_dram_parameter("out", [M, ntok, D], mybir.dt.float32, isOutput=True)

    with tile.TileContext(nc) as tc, ExitStack() as ctx:
        const_pool = ctx.enter_context(tc.tile_pool(name="const", bufs=1))
        base_pool = ctx.enter_context(tc.tile_pool(name="base", bufs=3))
        a_pool = ctx.enter_context(tc.tile_pool(name="a", bufs=3))
        at_pool = ctx.enter_context(tc.tile_pool(name="at", bufs=3))
        out_pool = ctx.enter_context(tc.tile_pool(name="outm", bufs=8))
        psum_t_pool = ctx.enter_context(tc.tile_pool(name="psum_t", bufs=2, space="PSUM"))
        psum_d_pool = ctx.enter_context(tc.tile_pool(name="psum_d", bufs=6, space="PSUM"))

        identity = const_pool.tile([P, P], mybir.dt.float32)
        make_identity(nc, identity[:])
        idx_sb = const_pool.tile([P, ntile], mybir.dt.int32)
        nc.sync.dma_start(out=idx_sb[:], in_=idx[:])
        # B^T per member, each tile on partitions 0..15 so matmul operands
        # start at partition 0.
        bt_sbs = []
        for m in range(M):
            bt_m = const_pool.tile([R, D], mm_dtype, tag=f"bt{m}")
            nc.sync.dma_start(out=bt_m[:], in_=bt[m])
            bt_sbs.append(bt_m)

        def mm_ap(ap):
            return ap.bitcast(mybir.dt.float32r) if use_f32r else ap

        for t in range(ntile):
            base_t = base_pool.tile([P, D], mybir.dt.float32, tag="base")
            nc.gpsimd.indirect_dma_start(
                out=base_t[:],
                out_offset=None,
                in_=w[:],
                in_offset=bass.IndirectOffsetOnAxis(ap=idx_sb[:, t : t + 1], axis=0),
            )
            a_t = a_pool.tile([P, M * R], mybir.dt.float32, tag="a")
            nc.gpsimd.indirect_dma_start(
                out=a_t[:],
                out_offset=None,
                in_=acat[:],
                in_offset=bass.IndirectOffsetOnAxis(ap=idx_sb[:, t : t + 1], axis=0),
            )
            # after_A for all members: 4 transposes [128,16] -> [16,128],
            # packed into one [16, 4*128] psum tile / sbuf tile.
            at_ps = psum_t_pool.tile([R, M * P], mybir.dt.float32, tag="at_ps")
            for m in range(M):
                nc.tensor.transpose(
                    out=at_ps[:, m * P : (m + 1) * P],
                    in_=a_t[:, m * R : (m + 1) * R],
                    identity=identity[:],
                )
            at_sb = at_pool.tile([R, M * P], mm_dtype, tag="at")
            nc.scalar.copy(out=at_sb[:], in_=at_ps[:])

            for m in range(M):
                out_m = out_pool.tile([P, D], mybir.dt.float32, tag="outm")
                for c in range(D // DCHUNK):
                    dps = psum_d_pool.tile([P, DCHUNK], mybir.dt.float32, tag="dps")
                    nc.tensor.matmul(
                        out=dps[:],
                        lhsT=mm_ap(at_sb[:, m * P : (m + 1) * P]),
                        rhs=mm_ap(bt_sbs[m][:, c * DCHUNK : (c + 1) * DCHUNK]),
                        start=True,
                        stop=True,
                    )
                    nc.vector.tensor_add(
                        out=out_m[:, c * DCHUNK : (c + 1) * DCHUNK],
                        in0=base_t[:, c * DCHUNK : (c + 1) * DCHUNK],
                        in1=dps[:],
                    )
                nc.sync.dma_start(out=out[m, t * P : (t + 1) * P, :], in_=out_m[:])

    nc.finalize()
    return nc


def _get_program(key=("default",)):
    if key not in _CACHE:
        _CACHE[key] = build_bass()
    return _CACHE[key]


def make_in_maps(x, weight, lora_A, lora_B):
    x = np.asarray(x).astype(np.int32).reshape(-1)                    # [B*T]
    weight = np.ascontiguousarray(np.asarray(weight, dtype=np.float32))
    # A_cat[v, m*R + r] = lora_A[m, r, v]
    acat = np.ascontiguousarray(
        np.asarray(lora_A, dtype=np.float32).transpose(2, 0, 1).reshape(V, M * R)
    )
    # bt[m, r, d] = lora_B[m, d, r]
    bt = np.ascontiguousarray(
        np.asarray(lora_B, dtype=np.float32).transpose(0, 2, 1)
    )

    in_maps = []
    for c in range(NCORES):
        xc = x[c * NTOK : (c + 1) * NTOK]
        idx = np.ascontiguousarray(xc.reshape(NTILE, P).T)            # [128, NTILE]
        in_maps.append({"w": weight, "acat": acat, "bt": bt, "idx": idx})
    return in_maps


def assemble(results):
    full = np.concatenate([r["out"] for r in results], axis=1)        # [M, B*T, D]
    return full.reshape(M, B, T, D)


def kernel(x, weight, lora_A, lora_B):
    in_maps = make_in_maps(x, weight, lora_A, lora_B)
    nc = _get_program()
    res = run_bass_kernel_spmd(nc, in_maps, core_ids=list(range(NCORES))).results
    return assemble(res)
